# revision 25
# baseline (speedup 1.0000x reference)
"""Trainium2 Bass kernel for nn_DNC_65189013619263.

6-layer transformer (D=1024, H=16, FF=4096, T=2048, B=2) + StateBank
content-addressed read + tied LM head over V=32000, run SPMD on 8
NeuronCores.

Sharding: 8-way over tokens (core c -> batch c//4, tokens (c%4)*512..+512).
k/v shards are AllGather'd within each batch's 4-core group (split by head
halves so attention can start early); weights replicated per core.

v2 design:
- Residual stream kept FEATURE-major in SBUF ([128 feat, DT, 512 tok]) --
  no PE transposes anywhere; rmsnorm partition-reductions via f32
  ones-matmuls.
- RoPE'd embeddings precomputed host-side, uploaded feature-major.
- Attention: per head pair, the two heads' score matmuls are row-tiled
  (partitions 0-63 / 64-127) and issued adjacently so they run
  concurrently in the PE array; AV matmuls are col-tiled (psum partitions
  0-63 / 64-127). Exp is batched 2 tiles per ACT call from PSUM; softmax
  denominators via DVE adds + ones-matmul broadcast + full-width DVE
  reciprocal (no 1-partition ops, no gpsimd broadcast).
- Local keys are processed first (from SBUF) so score compute starts
  before the AllGathers land; remote shards are fetched with per-core
  indirect-DMA offset tables.
"""

import os
import sys

sys.path.insert(0, "/opt/trn_rl_repo")

import numpy as np
import ml_dtypes

import concourse.bass as bass
import concourse.bacc as bacc
import concourse.tile as tile
import concourse.mybir as mybir
from concourse.bass import ds

V, D, L, H, SLOTS, FF = 32000, 1024, 6, 16, 4096, 4096
DH = D // H
B, T, NCORES, TLOC = 2, 2048, 8, 512
P = 128
DT = D // P            # 8 feature tiles
JT = TLOC // P         # 4 local token tiles
KT = T // P            # 16 key tiles (full batch)
FT = FF // P           # 32 ff tiles
ST = SLOTS // P        # 32 slot tiles
EPS = 1e-8

BF = mybir.dt.bfloat16
F32 = mybir.dt.float32
I32 = mybir.dt.int32
MUL = mybir.AluOpType.mult
ADD = mybir.AluOpType.add
SUB = mybir.AluOpType.subtract
AF = mybir.ActivationFunctionType
GROUPS = [[0, 1, 2, 3], [4, 5, 6, 7]]

L_RUN = int(os.environ.get("DNC_LAYERS", str(L)))
STAGE = os.environ.get("DNC_STAGE", "full")  # emb | layers | sb | full


class _Ctx:
    """Holds persistent tiles and pools during program build."""
    pass


# ----------------------------------------------------------------------------
# bass program
# ----------------------------------------------------------------------------

def _rmsnorm_fm(nc, tc, cx, xr, nwT):
    """Feature-major rmsnorm: x_res/xT = rmsnorm(xr) * nw.

    xr: [128, DT, 512] f32. Partition-axis sum of squares via f32
    ones-matmul into a broadcast [128, 512] psum.
    """
    with tc.tile_pool(name="nrm", bufs=1, space="PSUM") as pn:
        ssq = pn.tile([P, TLOC], F32, tag="ssq")
        for m in range(DT):
            sq = cx.sqp.tile([P, TLOC], BF, tag="sq")
            nc.vector.tensor_tensor(out=sq[:], in0=xr[:, m, :], in1=xr[:, m, :], op=MUL)
            nc.tensor.matmul(out=ssq[:], lhsT=cx.ones_bf[:, :P], rhs=sq[:],
                             start=(m == 0), stop=(m == DT - 1))
        sd = cx.sqp.tile([P, TLOC], F32, tag="sd")
        nc.scalar.activation(out=sd[:], in_=ssq[:], func=AF.Ln,
                             scale=1.0 / D, bias=cx.eps_t[:, :1])
        rinv = cx.sqp.tile([P, TLOC], F32, tag="rinv")
        nc.scalar.activation(out=rinv[:], in_=sd[:], func=AF.Exp, scale=-0.5)
        for m in range(DT):
            nc.vector.tensor_tensor(out=cx.x_res[:, m, :], in0=xr[:, m, :],
                                    in1=rinv[:], op=MUL)
            if nwT is not None:
                nc.vector.tensor_scalar_mul(cx.x_res[:, m, :], cx.x_res[:, m, :],
                                            nwT[:, m:m + 1])
            nc.vector.tensor_copy(out=cx.xT[:, m, :], in_=cx.x_res[:, m, :])



def _attn_pairs(nc, cx, pB, pss, pso, psr, kT_all, v_all, s_exp):
    """Attention for all 8 head pairs: row-tiled score MM pairs, col-tiled
    AV MM pairs, 2-tile-batched exp, matmul-broadcast softmax denominators.
    Local key tiles are processed first so compute starts before the
    AllGathers land."""
    for mh in range(H // 2):
        half = mh // 4
        dh = mh % 4
        h0, h1 = 2 * mh, 2 * mh + 1
        ps_o = pso.tile([P, TLOC], F32, tag="av")
        ps_r = psr.tile([P, TLOC], F32, tag="rsum")
        es = [None] * KT
        for ikt in range(KT + 2):
            if ikt < KT:
                r, jj = ikt // JT, ikt % JT
                kap = lambda pi, r=r, jj=jj: kT_all[half][ds(pi * DH, DH), r, dh,
                                                          ds(jj * P, P)]
                ps_s = pss.tile([P, 1024], F32, tag="ss")
                nc.tensor.matmul(out=ps_s[:, 0:512], lhsT=kap(0),
                                 rhs=cx.qT[ds(0, DH), mh, :], start=True, stop=True)
                nc.tensor.matmul(out=ps_s[:, 512:1024], lhsT=kap(1),
                                 rhs=cx.qT[ds(DH, DH), mh, :], start=True, stop=True)
                e = cx.expp.tile([P, 1024], BF, tag="expT")
                nc.scalar.activation(out=e[:], in_=ps_s[:], func=AF.Exp, scale=s_exp)
                es[ikt] = e
            if ikt >= 2:
                k2 = ikt - 2
                r, jj = k2 // JT, k2 % JT
                vap = lambda h, r=r, jj=jj: v_all[half][:, r, jj, ds((h % 8) * DH, DH)]
                e = es[k2]
                es[k2] = None
                nc.tensor.matmul(out=ps_o[0:DH, :], lhsT=vap(h0), rhs=e[:, 0:512],
                                 start=(k2 == 0), stop=(k2 == KT - 1))
                nc.tensor.matmul(out=ps_o[DH:P, :], lhsT=vap(h1), rhs=e[:, 512:1024],
                                 start=(k2 == 0), stop=(k2 == KT - 1))
                nc.tensor.matmul(out=ps_r[0:DH, :], lhsT=cx.ones_bf[:, :DH],
                                 rhs=e[:, 0:512], start=(k2 == 0), stop=(k2 == KT - 1))
                nc.tensor.matmul(out=ps_r[DH:P, :], lhsT=cx.ones_bf[:, :DH],
                                 rhs=e[:, 512:1024], start=(k2 == 0), stop=(k2 == KT - 1))
        # drain psum to SBUF fast so next pair's accumulation groups can start;
        # reciprocal + normalize run off the critical path
        oU = pB.tile([P, TLOC], F32, tag="oU", bufs=2, name="oU")
        nc.vector.tensor_copy(out=oU[:], in_=ps_o[:])
        rU = pB.tile([P, TLOC], F32, tag="rU", bufs=2, name="rU")
        nc.vector.tensor_copy(out=rU[:], in_=ps_r[:])
        recB = pB.tile([P, TLOC], F32, tag="rec", bufs=2)
        nc.vector.reciprocal(out=recB[:], in_=rU[:])
        nc.vector.tensor_tensor(out=cx.oT[:, mh, :], in0=oU[:], in1=recB[:],
                                op=MUL)


def _layer(nc, tc, cx, A, l):
    ex = lambda **kw: tc.tile_pool(**kw)
    xT = cx.xT
    s_exp = DH ** -0.5

    # ---------------- phase A: q/k/v projections + split AllGathers
    pkv_cm = tc.tile_pool(name=f"kv{l}", bufs=1)
    pkv = pkv_cm.__enter__()
    cx.kT_loc = pkv.tile([P, DT, TLOC], BF, tag="kT_loc", name="kT_loc")
    cx.v_loc = pkv.tile([P, JT, D], BF, tag="v_loc", name="v_loc")
    kT_all = [pkv.tile([P, 4, 4, TLOC], BF, tag=f"ka{h}", name=f"ka{h}") for h in range(2)]
    v_all = [pkv.tile([P, 4, 4, 512], BF, tag=f"va{h}", name=f"va{h}") for h in range(2)]
    with ex(name=f"A{l}", bufs=1) as pA, \
         ex(name=f"Amm{l}", bufs=4, space="PSUM") as pmm:
        wk_t = pA.tile([P, DT, D], BF, tag="w", bufs=2)
        nc.sync.dma_start(wk_t[:], A["wk"].ap()[l].rearrange("(o p) n -> p o n", p=P))
        wv_t = pA.tile([P, DT, D], BF, tag="w", bufs=2)
        nc.sync.dma_start(wv_t[:], A["wv"].ap()[l].rearrange("(o p) n -> p o n", p=P))
        wq_t = pA.tile([P, DT, D], BF, tag="wq", bufs=1)
        nc.sync.dma_start(wq_t[:], A["wq"].ap()[l].rearrange("(o p) n -> p o n", p=P))
        for half in range(2):
            for m in range(4 * half, 4 * half + 4):
                pk = pmm.tile([P, TLOC], F32, tag="mm")
                for d in range(DT):
                    nc.tensor.matmul(out=pk[:], lhsT=wk_t[:, d, ds(m * P, P)],
                                     rhs=xT[:, d, :], start=(d == 0), stop=(d == DT - 1))
                nc.vector.tensor_copy(out=cx.kT_loc[:, m, :], in_=pk[:])
            k_in = cx.dramp.tile([P, 2048], BF, tag=f"k_in{half}", name=f"k_in{half}")
            nc.sync.dma_start(k_in[:].rearrange("p (o t) -> p o t", o=4),
                              cx.kT_loc[:, ds(half * 4, 4), :])
            k_out = cx.dramp.tile([4 * P, 2048], BF, tag=f"k_out{half}", name=f"k_out{half}")
            nc.gpsimd.collective_compute(
                "AllGather", mybir.AluOpType.bypass, replica_groups=GROUPS,
                ins=[k_in[:].opt()], outs=[k_out[:].opt()])
            nc.sync.dma_start(kT_all[half][:],
                              k_out[:].rearrange("(r p) (o t) -> p r o t", p=P, o=4))
            for j in range(JT):
                pv = pmm.tile([P, TLOC], F32, tag="mm")
                for d in range(DT):
                    nc.tensor.matmul(out=pv[:], lhsT=xT[:, d, ds(j * P, P)],
                                     rhs=wv_t[:, d, ds(half * 512, 512)],
                                     start=(d == 0), stop=(d == DT - 1))
                nc.vector.tensor_copy(out=cx.v_loc[:, j, ds(half * 512, 512)], in_=pv[:])
            v_in = cx.dramp.tile([P, 2048], BF, tag=f"v_in{half}", name=f"v_in{half}")
            nc.sync.dma_start(v_in[:].rearrange("p (j f) -> p j f", j=JT),
                              cx.v_loc[:, :, ds(half * 512, 512)])
            v_out = cx.dramp.tile([4 * P, 2048], BF, tag=f"v_out{half}", name=f"v_out{half}")
            nc.gpsimd.collective_compute(
                "AllGather", mybir.AluOpType.bypass, replica_groups=GROUPS,
                ins=[v_in[:].opt()], outs=[v_out[:].opt()])
            nc.sync.dma_start(v_all[half][:],
                              v_out[:].rearrange("(r p) (j f) -> p r j f", p=P, j=JT))


        for m in range(DT):
            pq = pmm.tile([P, TLOC], F32, tag="mm")
            for d in range(DT):
                nc.tensor.matmul(out=pq[:], lhsT=wq_t[:, d, ds(m * P, P)],
                                 rhs=xT[:, d, :], start=(d == 0), stop=(d == DT - 1))
            nc.vector.tensor_copy(out=cx.qT[:, m, :], in_=pq[:])


    # ---------------- phase B: attention (pair-concurrent MMs) + out-proj
    with ex(name=f"B{l}", bufs=1) as pB:
        wo_t = pB.tile([P, DT, D], BF, tag="wo")
        nc.sync.dma_start(wo_t[:], A["wo"].ap()[l].rearrange("(o p) n -> p o n", p=P))
        with ex(name=f"Bs{l}", bufs=3, space="PSUM") as pss, \
             ex(name=f"Bo{l}", bufs=1, space="PSUM") as pso, \
             ex(name=f"Br{l}", bufs=1, space="PSUM") as psr:
            _attn_pairs(nc, cx, pB, pss, pso, psr, kT_all, v_all, s_exp)

        with ex(name=f"Bp{l}", bufs=3, space="PSUM") as pmm2:
            for m in range(DT):
                px = pmm2.tile([P, TLOC], F32, tag="mm")
                for d in range(DT):
                    nc.tensor.matmul(out=px[:], lhsT=wo_t[:, d, ds(m * P, P)],
                                     rhs=cx.oT[:, d, :], start=(d == 0), stop=(d == DT - 1))
                nc.vector.tensor_tensor(out=cx.x_res[:, m, :], in0=px[:],
                                        in1=cx.x_res[:, m, :], op=ADD)
                if cx.has_bo:
                    nc.vector.tensor_scalar_add(cx.x_res[:, m, :], cx.x_res[:, m, :],
                                                cx.boT[l][:, m:m + 1])
        _rmsnorm_fm(nc, tc, cx, cx.x_res, cx.n1T[l] if cx.has_n1 else None)
    pkv_cm.__exit__(None, None, None)

    # ---------------- phase C: FFN
    with ex(name=f"C{l}", bufs=1) as pC:
        su_full = pC.tile([P, FT, TLOC], BF, tag="su")
        wg_r = A["wg"].ap()[l].rearrange("(o p) f -> p o f", p=P)
        wu_r = A["wu"].ap()[l].rearrange("(o p) f -> p o f", p=P)
        with ex(name=f"Cg{l}", bufs=4, space="PSUM") as pmm:
            for f in range(FT):
                wg_t = pC.tile([P, DT, P], BF, tag="wff", bufs=12)
                nc.sync.dma_start(wg_t[:], wg_r[:, :, ds(f * P, P)])
                wu_t = pC.tile([P, DT, P], BF, tag="wff", bufs=12)
                nc.sync.dma_start(wu_t[:], wu_r[:, :, ds(f * P, P)])
                ps_g = pmm.tile([P, TLOC], F32, tag="mm")
                for d in range(DT):
                    nc.tensor.matmul(out=ps_g[:], lhsT=wg_t[:, d, :], rhs=xT[:, d, :],
                                     start=(d == 0), stop=(d == DT - 1))
                g_sb = pC.tile([P, TLOC], F32, tag="g_sb", bufs=3)
                nc.scalar.activation(out=g_sb[:], in_=ps_g[:], func=AF.Silu)
                ps_u = pmm.tile([P, TLOC], F32, tag="mm")
                for d in range(DT):
                    nc.tensor.matmul(out=ps_u[:], lhsT=wu_t[:, d, :], rhs=xT[:, d, :],
                                     start=(d == 0), stop=(d == DT - 1))
                nc.vector.tensor_tensor(out=su_full[:, f, :], in0=g_sb[:],
                                        in1=ps_u[:], op=MUL)
        with ex(name=f"Cd{l}", bufs=1, space="PSUM") as pacc:
            ps_d = [pacc.tile([P, TLOC], F32, tag=f"acc{m}", name=f"ps_d{m}")
                    for m in range(DT)]
            for f in range(FT):
                wd_t = pC.tile([P, D], BF, tag="wd", bufs=8)
                nc.sync.dma_start(wd_t[:], A["wd"].ap()[l][ds(f * P, P), :])
                for m in range(DT):
                    nc.tensor.matmul(out=ps_d[m][:], lhsT=wd_t[:, ds(m * P, P)],
                                     rhs=su_full[:, f, :], start=(f == 0),
                                     stop=(f == FT - 1))
            for m in range(DT):
                nc.vector.tensor_tensor(out=cx.x_res[:, m, :], in0=ps_d[m][:],
                                        in1=cx.x_res[:, m, :], op=ADD)
        _rmsnorm_fm(nc, tc, cx, cx.x_res, cx.n2T[l] if cx.has_n2 else None)


def _statebank(nc, tc, cx, A):
    ex = lambda **kw: tc.tile_pool(**kw)
    xT = cx.xT
    with ex(name="S", bufs=1) as pS:
        wsp_t = pS.tile([P, DT, D], BF, tag="wsp")
        nc.sync.dma_start(wsp_t[:], A["wsp"].ap().rearrange("(o p) n -> p o n", p=P))
        with ex(name="Sq", bufs=4, space="PSUM") as pmm:
            for m in range(DT):
                ps_q = pmm.tile([P, TLOC], F32, tag="mm")
                for d in range(DT):
                    nc.tensor.matmul(out=ps_q[:], lhsT=wsp_t[:, d, ds(m * P, P)],
                                     rhs=xT[:, d, :], start=(d == 0), stop=(d == DT - 1))
                if cx.has_bsp:
                    nc.scalar.activation(out=cx.qT[:, m, :], in_=ps_q[:],
                                         func=AF.Identity, bias=cx.bspT[:, m:m + 1])
                else:
                    nc.vector.tensor_copy(out=cx.qT[:, m, :], in_=ps_q[:])

        expS = pS.tile([P, ST // 2, 1024], BF, tag="sexp")
        memT_r = A["memT"].ap().rearrange("(o p) s -> p o s", p=P)
        with ex(name="Ss", bufs=3, space="PSUM") as pss, \
             ex(name="Ssr", bufs=1, space="PSUM") as pssr:
            ps_rS = pssr.tile([P, TLOC], F32, tag="rsum")
            for sb2 in range(ST // 2):
                ps_s = pss.tile([P, 1024], F32, tag="ss")
                for i in range(2):
                    s = 2 * sb2 + i
                    mt_t = pS.tile([P, DT, P], BF, tag="mt", bufs=6)
                    nc.sync.dma_start(mt_t[:], memT_r[:, :, ds(s * P, P)])
                    for d in range(DT):
                        nc.tensor.matmul(out=ps_s[:, ds(i * 512, 512)],
                                         lhsT=mt_t[:, d, :], rhs=cx.qT[:, d, :],
                                         start=(d == 0), stop=(d == DT - 1))
                nc.scalar.activation(out=expS[:, sb2, :], in_=ps_s[:], func=AF.Exp,
                                     scale=D ** -0.5)
                nc.tensor.matmul(out=ps_rS[:], lhsT=cx.ones_bf[:, :P],
                                 rhs=expS[:, sb2, 0:512],
                                 start=(sb2 == 0), stop=False)
                nc.tensor.matmul(out=ps_rS[:], lhsT=cx.ones_bf[:, :P],
                                 rhs=expS[:, sb2, 512:1024],
                                 start=False, stop=(sb2 == ST // 2 - 1))
            lnS = pS.tile([P, TLOC], F32, tag="lnS")
            nc.scalar.activation(out=lnS[:], in_=ps_rS[:], func=AF.Ln)
            recS = pS.tile([P, TLOC], F32, tag="recS")
            nc.scalar.activation(out=recS[:], in_=lnS[:], func=AF.Exp, scale=-1.0)

        rT = cx.oT  # reuse [128, DT, 512] bf16
        with ex(name="Sd", bufs=1, space="PSUM") as pacc:
            ps_rd = [pacc.tile([P, TLOC], F32, tag=f"acc{m}", name=f"ps_rd{m}")
                     for m in range(DT)]
            for s in range(ST):
                mb_t = pS.tile([P, D], BF, tag="mb", bufs=6)
                nc.sync.dma_start(mb_t[:], A["memB"].ap()[ds(s * P, P), :])
                for m in range(DT):
                    nc.tensor.matmul(out=ps_rd[m][:], lhsT=mb_t[:, ds(m * P, P)],
                                     rhs=expS[:, s // 2, ds((s % 2) * 512, 512)],
                                     start=(s == 0), stop=(s == ST - 1))
            for m in range(DT):
                nc.vector.tensor_tensor(out=rT[:, m, :], in0=ps_rd[m][:],
                                        in1=recS[:], op=MUL)

        wrp_t = pS.tile([P, DT, D], BF, tag="wsp")
        nc.sync.dma_start(wrp_t[:], A["wrp"].ap().rearrange("(o p) n -> p o n", p=P))
        with ex(name="Sp", bufs=3, space="PSUM") as pmm2:
            for m in range(DT):
                px = pmm2.tile([P, TLOC], F32, tag="mm")
                for d in range(DT):
                    nc.tensor.matmul(out=px[:], lhsT=wrp_t[:, d, ds(m * P, P)],
                                     rhs=rT[:, d, :], start=(d == 0), stop=(d == DT - 1))
                nc.vector.tensor_tensor(out=cx.x_res[:, m, :], in0=px[:],
                                        in1=cx.x_res[:, m, :], op=ADD)
                if cx.has_brp:
                    nc.vector.tensor_scalar_add(cx.x_res[:, m, :], cx.x_res[:, m, :],
                                                cx.brpT[:, m:m + 1])
        _rmsnorm_fm(nc, tc, cx, cx.x_res, cx.noutT if cx.has_nout else None)


def _lm_head(nc, tc, cx, A, out_t):
    ex = lambda **kw: tc.tile_pool(**kw)
    NV = (V + 511) // 512  # 63: 62*512 + 256
    with ex(name="LM", bufs=1) as pL, \
         ex(name="Lacc", bufs=6, space="PSUM") as pacc:
        et_r = A["et"].ap().rearrange("(o p) v -> p o v", p=P)
        for vt in range(NV):
            nv = 512 if vt < NV - 1 else V - 512 * (NV - 1)
            et_t = pL.tile([P, DT, 512], BF, tag="et", bufs=4)
            nc.sync.dma_start(et_t[:, :, :nv], et_r[:, :, ds(vt * 512, nv)])
            for j in range(JT):
                ps = pacc.tile([P, 512], F32, tag="acc")
                for d in range(DT):
                    nc.tensor.matmul(out=ps[:, :nv], lhsT=cx.xT[:, d, ds(j * P, P)],
                                     rhs=et_t[:, d, :nv], start=(d == 0), stop=(d == DT - 1))
                lg_sb = pL.tile([P, 512], BF, tag="lg", bufs=6)
                nc.vector.tensor_copy(out=lg_sb[:, :nv], in_=ps[:, :nv])
                nc.sync.dma_start(out_t.ap()[ds(j * P, P), ds(vt * 512, nv)],
                                  lg_sb[:, :nv])


def _prog(nc, tc, A, out_t, flags):
    ex = lambda **kw: tc.tile_pool(**kw)
    cx = _Ctx()
    for k, v in flags.items():
        setattr(cx, k, v)
    with ex(name="const", bufs=1) as constp, \
         ex(name="xres", bufs=1) as xresp, \
         ex(name="xT", bufs=1) as xTp, \
         ex(name="qT", bufs=1) as qTp, \
         ex(name="oT", bufs=1) as oTp, \
         ex(name="exp", bufs=6) as cx.expp, \
         ex(name="sq", bufs=3) as cx.sqp, \
         ex(name="nw", bufs=1) as nwp, \
         ex(name="dram", bufs=2, space="DRAM") as cx.dramp:
        cx.ones_f = constp.tile([P, P], F32)
        nc.any.memset(cx.ones_f[:], 1.0)
        cx.ones_bf = constp.tile([P, P], BF)
        nc.any.memset(cx.ones_bf[:], 1.0)
        cx.eps_t = constp.tile([P, 1], F32)
        nc.any.memset(cx.eps_t[:], EPS)
        cx.x_res = xresp.tile([P, DT, TLOC], F32)
        cx.xT = xTp.tile([P, DT, TLOC], BF)
        cx.qT = qTp.tile([P, DT, TLOC], BF)
        cx.oT = oTp.tile([P, DT, TLOC], BF)
        if L_RUN > 0:
            cx.n1T, cx.n2T, cx.boT = {}, {}, {}
            for l in range(L_RUN):
                if cx.has_n1:
                    cx.n1T[l] = nwp.tile([P, DT], F32, tag=f"n1T{l}")
                    nc.sync.dma_start(cx.n1T[l][:], A["n1T"].ap()[l])
                if cx.has_n2:
                    cx.n2T[l] = nwp.tile([P, DT], F32, tag=f"n2T{l}")
                    nc.sync.dma_start(cx.n2T[l][:], A["n2T"].ap()[l])
                if cx.has_bo:
                    cx.boT[l] = nwp.tile([P, DT], F32, tag=f"boT{l}")
                    nc.sync.dma_start(cx.boT[l][:], A["boT"].ap()[l])
        if STAGE in ("sb", "full"):
            if cx.has_bsp:
                cx.bspT = nwp.tile([P, DT], F32, tag="bspT")
                nc.sync.dma_start(cx.bspT[:], A["bspT"].ap())
            if cx.has_brp:
                cx.brpT = nwp.tile([P, DT], F32, tag="brpT")
                nc.sync.dma_start(cx.brpT[:], A["brpT"].ap())
            if cx.has_nout:
                cx.noutT = nwp.tile([P, DT], F32, tag="noutT")
                nc.sync.dma_start(cx.noutT[:], A["noutT"].ap())

        # ------------------------------------------------- embedding (host-roped)
        nc.sync.dma_start(cx.x_res[:], A["x0T"].ap().rearrange("(o p) t -> p o t", p=P))
        for m in range(DT):
            nc.vector.tensor_copy(out=cx.xT[:, m, :], in_=cx.x_res[:, m, :])

        if STAGE == "emb":
            for m in range(DT):
                nc.sync.dma_start(out_t.ap()[:, ds(m * P, P)].rearrange("t p -> p t"),
                                  cx.x_res[:, m, :])
            return

        for l in range(L_RUN):
            _layer(nc, tc, cx, A, l)

        if STAGE == "layers":
            for m in range(DT):
                nc.sync.dma_start(out_t.ap()[:, ds(m * P, P)].rearrange("t p -> p t"),
                                  cx.x_res[:, m, :])
            return

        _statebank(nc, tc, cx, A)

        if STAGE == "sb":
            for m in range(DT):
                nc.sync.dma_start(out_t.ap()[:, ds(m * P, P)].rearrange("t p -> p t"),
                                  cx.x_res[:, m, :])
            return

        _lm_head(nc, tc, cx, A, out_t)


def build(flags):
    nc = bacc.Bacc("TRN2", target_bir_lowering=False, debug=False,
                   num_devices=NCORES)
    A = {}

    def inp(name, shape, dt):
        A[name] = nc.dram_tensor(name, list(shape), dt, kind="ExternalInput")
        return A[name]

    inp("x0T", (D, TLOC), F32)
    if L_RUN > 0:
        inp("wq", (L_RUN, D, D), BF)
        inp("wk", (L_RUN, D, D), BF)
        inp("wv", (L_RUN, D, D), BF)
        inp("wo", (L_RUN, D, D), BF)
        if flags["has_bo"]:
            inp("boT", (L_RUN, P, DT), F32)
        if flags["has_n1"]:
            inp("n1T", (L_RUN, P, DT), F32)
        inp("wg", (L_RUN, D, FF), BF)
        inp("wu", (L_RUN, D, FF), BF)
        inp("wd", (L_RUN, FF, D), BF)
        if flags["has_n2"]:
            inp("n2T", (L_RUN, P, DT), F32)
    if STAGE in ("sb", "full"):
        inp("wsp", (D, D), BF)
        if flags["has_bsp"]:
            inp("bspT", (P, DT), F32)
        inp("memT", (D, SLOTS), BF)
        inp("memB", (SLOTS, D), BF)
        inp("wrp", (D, D), BF)
        if flags["has_brp"]:
            inp("brpT", (P, DT), F32)
        if flags["has_nout"]:
            inp("noutT", (P, DT), F32)
    if STAGE == "full":
        inp("et", (D, V), BF)
        # bf16 logits halve the 65.5MB/core output write; host upcasts to f32
        out_t = nc.dram_tensor("logits", [TLOC, V], BF, kind="ExternalOutput")
    else:
        out_t = nc.dram_tensor("xdbg", [TLOC, D], F32, kind="ExternalOutput")

    with tile.TileContext(nc) as tc:
        _prog(nc, tc, A, out_t, flags)
    nc.compile()
    return nc


# ----------------------------------------------------------------------------
# host-side input prep
# ----------------------------------------------------------------------------

def _bf(x):
    return np.ascontiguousarray(np.asarray(x, dtype=np.float32).astype(ml_dtypes.bfloat16))


def _f32(x):
    return np.ascontiguousarray(np.asarray(x, dtype=np.float32))


def _fm(a):
    """[.., D] per-feature vec -> [.., P, DT] feature-major (f = o*P + p)."""
    a = np.asarray(a, np.float32)
    return np.ascontiguousarray(a.reshape(*a.shape[:-1], DT, P).swapaxes(-1, -2))


def _rope_host(x, pos):
    """x: [T, D] f32, pos: [T] -> interleaved RoPE, matching reference."""
    inv_freq = 1.0 / (10000.0 ** (np.arange(0, D, 2, dtype=np.float32) / D))
    freqs = pos[:, None].astype(np.float32) * inv_freq[None, :]
    cos, sin = np.cos(freqs), np.sin(freqs)
    x1, x2 = x[:, ::2], x[:, 1::2]
    out = np.empty_like(x)
    out[:, ::2] = x1 * cos - x2 * sin
    out[:, 1::2] = x1 * sin + x2 * cos
    return out


def _prep(inputs):
    ids = np.asarray(inputs["ids"]).astype(np.int64)
    tok_embed = np.asarray(inputs["tok_embed"], dtype=np.float32)
    triv = lambda a, v: bool(np.all(np.asarray(a) == v))

    flags = {
        "has_bo": not triv(inputs["bo"], 0.0),
        "has_n1": not triv(inputs["n1"], 1.0),
        "has_n2": not triv(inputs["n2"], 1.0),
        "has_bsp": not triv(inputs["bsp"], 0.0),
        "has_brp": not triv(inputs["brp"], 0.0),
        "has_nout": not triv(inputs["nout"], 1.0),
    }

    shared = {}
    if L_RUN > 0:
        shared.update({
            "wq": _bf(inputs["Wq"][:L_RUN]),
            "wk": _bf(inputs["Wk"][:L_RUN]),
            "wv": _bf(inputs["Wv"][:L_RUN]),
            "wo": _bf(inputs["Wo"][:L_RUN]),
            "wg": _bf(inputs["Wg"][:L_RUN]),
            "wu": _bf(inputs["Wu"][:L_RUN]),
            "wd": _bf(inputs["Wd"][:L_RUN]),
        })
        if flags["has_bo"]:
            shared["boT"] = _fm(np.asarray(inputs["bo"], np.float32)[:L_RUN])
        if flags["has_n1"]:
            shared["n1T"] = _fm(np.asarray(inputs["n1"], np.float32)[:L_RUN])
        if flags["has_n2"]:
            shared["n2T"] = _fm(np.asarray(inputs["n2"], np.float32)[:L_RUN])
    if STAGE in ("sb", "full"):
        mem = np.asarray(inputs["mem"], np.float32)
        shared.update({
            "wsp": _bf(inputs["Wsp"]),
            "memT": _bf(mem.T),
            "memB": _bf(mem),
            "wrp": _bf(inputs["Wrp"]),
        })
        if flags["has_bsp"]:
            shared["bspT"] = _fm(inputs["bsp"])
        if flags["has_brp"]:
            shared["brpT"] = _fm(inputs["brp"])
        if flags["has_nout"]:
            shared["noutT"] = _fm(inputs["nout"])
    if STAGE == "full":
        shared["et"] = _bf(tok_embed.T)

    percore = []
    for c in range(NCORES):
        b, s = c // 4, c % 4
        tok = ids[b, s * TLOC:(s + 1) * TLOC]
        pos = s * TLOC + np.arange(TLOC)
        x0 = _rope_host(tok_embed[tok], pos)
        pc = {"x0T": np.ascontiguousarray(x0.T, np.float32)}
        percore.append(pc)
    return shared, percore, flags


# ----------------------------------------------------------------------------
# runner: shared arrays uploaded once + device-to-device fanout
# ----------------------------------------------------------------------------

def _run_fast(nc, shared, percore, n_cores=NCORES):
    import jax
    from jax.sharding import Mesh, PartitionSpec, NamedSharding
    from jax.experimental.shard_map import shard_map
    from concourse import bass2jax

    bass2jax.install_neuronx_cc_hook()
    devs = jax.devices()[:n_cores]
    mesh = Mesh(np.asarray(devs), ("core",))
    spec = PartitionSpec("core")
    shd = NamedSharding(mesh, spec)

    placed = {}
    for name, arr in shared.items():
        a0 = jax.device_put(arr, devs[0])
        a0.block_until_ready()
        reps = [a0] + [jax.device_put(a0, d) for d in devs[1:]]
        gshape = (n_cores * arr.shape[0], *arr.shape[1:])
        placed[name] = jax.make_array_from_single_device_arrays(gshape, shd, reps)
    for name in percore[0]:
        arrs = [jax.device_put(percore[c][name], devs[c]) for c in range(n_cores)]
        a = percore[0][name]
        gshape = (n_cores * a.shape[0], *a.shape[1:])
        placed[name] = jax.make_array_from_single_device_arrays(gshape, shd, arrs)

    partition_name = nc.partition_id_tensor.name if nc.partition_id_tensor else None
    in_names, out_names, out_avals = [], [], []
    for alloc in nc.m.functions[0].allocations:
        if not isinstance(alloc, mybir.MemoryLocationSet):
            continue
        name = alloc.memorylocations[0].name
        if alloc.kind == "ExternalInput":
            if name != partition_name:
                in_names.append(name)
        elif alloc.kind == "ExternalOutput":
            out_names.append(name)
            out_avals.append(jax.core.ShapedArray(tuple(alloc.tensor_shape),
                                                  mybir.dt.np(alloc.dtype)))
    n_params = len(in_names)
    all_in_names = list(in_names) + list(out_names)
    if partition_name is not None:
        all_in_names.append(partition_name)

    def _body(*args):
        operands = list(args)
        if partition_name is not None:
            operands.append(bass2jax.partition_id_tensor())
        outs = bass2jax._bass_exec_p.bind(
            *operands,
            out_avals=tuple(out_avals),
            in_names=tuple(all_in_names),
            out_names=tuple(out_names),
            lowering_input_output_aliases=(),
            sim_require_finite=True,
            sim_require_nnan=True,
            nc=nc,
        )
        return tuple(outs)

    profile_req = bool(os.environ.get("DNC_PROF"))
    donate = tuple(range(n_params, n_params + len(out_names)))
    sharded = jax.jit(
        shard_map(_body, mesh=mesh, in_specs=(spec,) * (n_params + len(out_names)),
                  out_specs=(spec,) * len(out_names), check_rep=False),
        donate_argnums=donate, keep_unused=True)

    def mk_zeros():
        return [
            jax.jit(lambda av=av: jax.numpy.zeros((n_cores * av.shape[0], *av.shape[1:]), av.dtype),
                    out_shardings=shd)()
            for av in out_avals
        ]

    args = [placed[name] for name in in_names] + mk_zeros()
    out_arrs = sharded(*args)
    bench_n = int(os.environ.get("DNC_BENCH", "0"))
    if bench_n:
        import time as _time
        [o.block_until_ready() for o in out_arrs]
        ts = []
        for _ in range(bench_n):
            t0 = _time.perf_counter()
            outs2 = sharded(*([placed[name] for name in in_names] + mk_zeros()))
            [o.block_until_ready() for o in outs2]
            ts.append(_time.perf_counter() - t0)
        print(f"bench wall ms: {[round(t * 1e3, 2) for t in ts]}; min {min(ts) * 1e3:.2f}")
    if profile_req:
        [o.block_until_ready() for o in out_arrs]
        _profile_exec(nc, lambda: sharded(*([placed[name] for name in in_names] + mk_zeros())))
    res = []
    for c in range(n_cores):
        res.append({
            name: np.asarray(out_arrs[i]).reshape(n_cores, *out_avals[i].shape)[c]
            for i, name in enumerate(out_names)
        })
    return res


def _get_ntff_hook(so_path="/opt/axon/libaxon_pjrt.so"):
    """Ctypes replica of axon's NTFF profile hook (antenv.axon_hooks is
    absent in this image; the .so C ABI is stable)."""
    import contextlib
    import ctypes
    try:
        lib = ctypes.CDLL(so_path)
    except OSError:
        return None
    if not hasattr(lib, "axon_start_nrt_profile"):
        return None
    lib.axon_start_nrt_profile.argtypes = [ctypes.POINTER(ctypes.c_int64),
                                           ctypes.c_size_t]
    lib.axon_start_nrt_profile.restype = ctypes.c_int64
    lib.axon_stop_nrt_profile.argtypes = [ctypes.c_char_p]
    lib.axon_stop_nrt_profile.restype = ctypes.c_int64

    @contextlib.contextmanager
    def _hook(output_dir, device_ids):
        import jax
        jax.devices()
        if device_ids:
            ids = (ctypes.c_int64 * len(device_ids))(*device_ids)
            rc = lib.axon_start_nrt_profile(ids, len(device_ids))
        else:
            rc = lib.axon_start_nrt_profile(None, 0)
        if rc != 0:
            raise RuntimeError(f"axon_start_nrt_profile rc={rc}")
        try:
            yield
        finally:
            n = lib.axon_stop_nrt_profile(str(output_dir).encode())
            print(f"profile: {n} file(s) written to {output_dir}", file=sys.stderr)

    return _hook


def _profile_exec(nc, run_fn):
    """Re-run the jitted NEFF under the axon NTFF hook; print HW exec time."""
    import glob as _glob
    import tempfile
    try:
        hook = _get_ntff_hook()
        if hook is None:
            print("HW exec time: unavailable (no ntff hook)")
            return
        import gauge.profiler
        from concourse import bass_utils as BU
        from concourse._compat import FishPath
        tmpdir = tempfile.mkdtemp(prefix="dnc_prof_")
        with hook(tmpdir, [0]):
            outs = run_fn()
            [o.block_until_ready() for o in outs]
        ntffs = _glob.glob(os.path.join(tmpdir, "*_body*.ntff"))
        if not ntffs:
            print("HW exec time: unavailable (no ntff produced)")
            return
        profile = gauge.profiler.Profile(
            profile_path=FishPath(tmpdir), kernel_dev_mode=True,
            profile_on_exit=False, bass_kernel=nc.m, offline_processing=True,
            fname="*_body*", metadata={"artifacts_path": tmpdir})
        res = BU._process_ntff_profile(
            profile, tmpdir, nc, list(range(NCORES)), None, False, {},
            trace_events=False)
        print(f"HW exec time: {res.exec_time_ns} ns")
        print(f"profile dir: {tmpdir}")
        if res.insts_and_trace_path:
            print(f"trace: {res.insts_and_trace_path[1]}")
    except Exception as e:
        import traceback; traceback.print_exc()
        print(f"HW exec time: error ({e})")


def _run_plain(nc, shared, percore, **kw):
    from concourse.bass_utils import run_bass_kernel_spmd
    in_maps = [dict(shared, **percore[c]) for c in range(NCORES)]
    return run_bass_kernel_spmd(nc, in_maps, core_ids=list(range(NCORES)), **kw)


_NC_CACHE = {}


def _get_nc(flags):
    key = (L_RUN, STAGE, tuple(sorted(flags.items())))
    if key not in _NC_CACHE:
        _NC_CACHE[key] = build(flags)
    return _NC_CACHE[key]


def _assemble(res, name, width):
    out = np.empty((B, T, width), np.float32)
    for c in range(NCORES):
        b, s = c // 4, c % 4
        out[b, s * TLOC:(s + 1) * TLOC, :] = res[c][name]
    return out


def kernel(**inputs):
    shared, percore, flags = _prep(inputs)
    nc = _get_nc(flags)
    if os.environ.get("DNC_PLAIN"):
        res = _run_plain(nc, shared, percore).results
    else:
        res = _run_fast(nc, shared, percore)
    if STAGE == "full":
        return _assemble(res, "logits", V)
    return _assemble(res, "xdbg", D)
